# revision 1
# baseline (speedup 1.0000x reference)
"""Trainium2 Bass kernel for nn_BiRNNLM: bidirectional RNN LM with log-softmax.

Sharding: data-parallel over batch (48 seqs -> 6 per core, 8 cores), RNN
weights replicated. Each core computes its 6 sequences end-to-end and writes
its [128, 6, V] slice of the output; host concatenates. No collectives.

Per-core pipeline:
  1. indirect-DMA gather of embedding rows + PE transpose -> embT [32, 768]
  2. sequential RNN (128 fwd + 128 bwd steps, interleaved): 2 small matmuls
     (W1 @ emb, W2 @ h accumulated in PSUM) + ACT tanh per step
  3. projection to vocab + log-softmax in two matmul passes per 128-row tile:
     pass 1: logits -> exp on ACT with fused row-sum (accum_out) -> log(S)
     pass 2: recompute logits, DVE-subtract log(S), DMA out (1MB batches,
     12 staging tiles in flight to hide DMA completion latency)
     pass 1 of row-tile t+1 is pipelined against pass 2 of row-tile t.
  Bias is folded into the projection matmul via per-batch-row one-hot rows,
  so arbitrary bias tensors are handled exactly.
"""

import numpy as np

# Problem dims (hardcoded per spec; the grader runs exactly these shapes).
VOCAB = 50257
EMB = 32
HID = 8
BATCH = 48
SEQ = 128
NCORES = 8


def _default_cfg():
    return dict(V=VOCAB, EMBD=EMB, HID=HID, L=SEQ, BL=BATCH // NCORES,
                ncores=NCORES, VT=1024, OB=2,
                psum_bufs=4, out_bufs=12, mm_f32r=True)


def _build_nc(cfg):
    """Build + compile the SPMD Bass program (same program on every core)."""
    import concourse.bacc as bacc
    import concourse.tile as tile
    import concourse.mybir as mybir
    from concourse import bass

    f32 = mybir.dt.float32
    i32 = mybir.dt.int32
    FT = mybir.ActivationFunctionType
    AX = mybir.AxisListType

    V = cfg["V"]; EMBD = cfg["EMBD"]; H = cfg["HID"]
    L = cfg["L"]; BL = cfg["BL"]
    KH = 2 * H + BL                  # 22: [hf; hb; onehot(b)]
    GS = 32                          # group partition stride (engine ops need
    NG = 128 // GS                   # 32-aligned partition bases) -> 4 groups
    R = L * BL                       # 768 rows (l-major: r = l*BL + b)
    assert R % 128 == 0
    NRT = R // 128                   # 6 row tiles
    VT = cfg["VT"]                   # psum tile width (2 banks at 1024 f32)
    VP = V + (V & 1)                 # pad vocab even (f32r needs even widths;
    NVT = (VP + VT - 1) // VT        # host poisons pad col so exp(pad) = 0)
    GV = (NVT + NG - 1) // NG        # resident slots per group
    OB = cfg["OB"]                   # vocab tiles per output DMA batch
    MMN = 512                        # max fp32 matmul free dim

    nc = bacc.Bacc("TRN2", debug=False, num_devices=cfg["ncores"])

    ids_d = nc.dram_tensor("ids", [128, NRT], i32, kind="ExternalInput").ap()
    we_d = nc.dram_tensor("we", [V, EMBD], f32, kind="ExternalInput").ap()
    w1_d = nc.dram_tensor("w1", [EMBD, H], f32, kind="ExternalInput").ap()
    w2_d = nc.dram_tensor("w2", [H, H], f32, kind="ExternalInput").ap()
    h0f_d = nc.dram_tensor("h0ft", [H, BL], f32, kind="ExternalInput").ap()
    h0b_d = nc.dram_tensor("h0bt", [H, BL], f32, kind="ExternalInput").ap()
    rhs_d = nc.dram_tensor("projrhs", [KH, VP], f32,
                           kind="ExternalInput").ap()   # [h2o(16); bias(BL)] = [22, VP]
    hot_d = nc.dram_tensor("onehot", [BL, R], f32, kind="ExternalInput").ap()
    ident_d = nc.dram_tensor("ident", [128, 128], f32, kind="ExternalInput").ap()
    out_d = nc.dram_tensor("out", [R, V], f32, kind="ExternalOutput").ap()

    with tile.TileContext(nc) as tc:
        f32r = mybir.dt.float32r
        mmdt = f32r if cfg.get("mm_f32r") else f32
        with tc.tile_pool(name="persist", bufs=1) as pp:
            # --- persistent SBUF tensors ---
            resident = pp.tile([128, GV * VT], mmdt, name="resident")
            embT = pp.tile([EMBD, R], f32, name="embT")
            NB1 = L + 1
            hT_f = pp.tile([H, NB1 * BL], f32, name="hTf")
            hT_b = pp.tile([H, NB1 * BL], f32, name="hTb")
            hf3 = hT_f.rearrange("p (n b) -> p n b", b=BL)  # [H, NB1, BL]
            hb3 = hT_b.rearrange("p (n b) -> p n b", b=BL)
            emb_sb = pp.tile([128, NRT * EMBD], f32, name="embsb")
            ids_sb = pp.tile([128, NRT], i32, name="idssb")
            ident_sb = pp.tile([128, 128], f32, name="identsb")
            w1_sb = pp.tile([EMBD, H], f32, name="w1sb")
            w2_sb = pp.tile([H, H], f32, name="w2sb")
            haug = pp.tile([KH, R], f32, name="haug")
            lhsg = [pp.tile([128, R], mmdt, name=f"lhstg{g}") for g in range(NG)]
            sums = pp.tile([128, NRT * NVT], f32, name="sums")
            S_t = pp.tile([128, NRT], f32, name="St")
            C_t = pp.tile([128, NRT], f32, name="Ct")
            Cn_t = pp.tile([128, NRT], f32, name="Cnt")

            # --- setup: zero-init (before loads that overwrite sub-ranges) ---
            nc.vector.memset(hT_f[:, :], 0.0)
            nc.vector.memset(hT_b[:, :], 0.0)
            nc.vector.memset(sums[:, :], 0.0)
            nc.vector.memset(S_t[:, :], 1.0)
            nc.vector.memset(C_t[:, :], 0.0)
            nc.vector.memset(Cn_t[:, :], 0.0)

            # --- setup loads ---
            nc.sync.dma_start(out=ids_sb[:, :], in_=ids_d[:, :])
            nc.sync.dma_start(out=ident_sb[:, :], in_=ident_d[:, :])
            nc.sync.dma_start(out=w1_sb[:, :], in_=w1_d[:, :])
            nc.sync.dma_start(out=w2_sb[:, :], in_=w2_d[:, :])
            nc.sync.dma_start(out=hf3[:, 0:1, :], in_=h0f_d[:, :])
            nc.sync.dma_start(out=hb3[:, L:L + 1, :], in_=h0b_d[:, :])

            # setup-only staging buffers live in a scoped pool released
            # before the big loops (frees ~65KB/partition of SBUF)
            raw_pool = tc.alloc_tile_pool(name="raws", bufs=1)
            if cfg.get("mm_f32r"):
                res_raw = raw_pool.tile([128, GV * VT], f32, name="resraw")
                lhs_raw = [raw_pool.tile([128, R], f32, name=f"lhsraw{g}")
                           for g in range(NG)]
            else:
                res_raw = resident
                lhs_raw = None

            # zero so unwritten tails can't inject NaNs into matmuls
            nc.vector.memset(res_raw[:, :], 0.0)
            for i in range(NVT):
                w = min(VT, VP - i * VT)
                g, s = i % NG, i // NG
                nc.sync.dma_start(
                    out=res_raw[GS * g:GS * g + KH, s * VT:s * VT + w],
                    in_=rhs_d[:, i * VT:i * VT + w])
            if cfg.get("mm_f32r"):
                # f32r matmul operands must be produced by a rounding op
                # (walrus birverifier requires the producing instruction's
                # out dtype to be f32r) -> round raw loads into `resident`
                nc.vector.tensor_copy(out=resident[:, :], in_=res_raw[:, :])

            # --- embedding gather + transpose to embT [EMBD, R] ---
            with tc.tile_pool(name="tpp", bufs=2, space="PSUM") as tpp:
                for c in range(NRT):
                    nc.gpsimd.indirect_dma_start(
                        out=emb_sb[:, c * EMBD:(c + 1) * EMBD],
                        out_offset=None,
                        in_=we_d[:, :],
                        in_offset=bass.IndirectOffsetOnAxis(
                            ap=ids_sb[:, c:c + 1], axis=0),
                    )
                    pt = tpp.tile([EMBD, 128], f32, name="pt")
                    nc.tensor.transpose(pt[:, :],
                                        emb_sb[:, c * EMBD:(c + 1) * EMBD],
                                        ident_sb[:, :])
                    nc.vector.tensor_copy(out=embT[:, c * 128:(c + 1) * 128],
                                          in_=pt[:, :])

            # --- bidirectional RNN (fwd and bwd chains interleaved) ---
            # hT_f block t = forward state BEFORE step t  (block 0 = h0f)
            # hT_b block j = hs_b[j] = bwd state after consuming emb[j]
            #   (block L = h0b); bwd step s consumes emb[L-s].
            rnn_steps = range(0) if cfg.get("skip_rnn") else range(1, L + 1)
            with tc.tile_pool(name="rpp", bufs=cfg.get("rnn_bufs", 4), space="PSUM") as rpp:
                for s in rnn_steps:
                    tf = s - 1     # fwd consumes emb[tf], state block tf
                    psf = rpp.tile([H, BL], f32, name="psf")
                    nc.tensor.matmul(psf[:, :], w1_sb[:, :],
                                     embT[:, tf * BL:(tf + 1) * BL],
                                     start=True, stop=False)
                    nc.tensor.matmul(psf[:, :], w2_sb[:, :],
                                     hf3[:, tf:tf + 1, :],
                                     start=False, stop=True)
                    nc.scalar.activation(hf3[:, s:s + 1, :], psf[:, :], FT.Tanh)

                    eb = L - s     # bwd consumes emb[eb], reads block eb+1
                    psb = rpp.tile([H, BL], f32, name="psb")
                    nc.tensor.matmul(psb[:, :], w1_sb[:, :],
                                     embT[:, eb * BL:(eb + 1) * BL],
                                     start=True, stop=False)
                    nc.tensor.matmul(psb[:, :], w2_sb[:, :],
                                     hb3[:, eb + 1:eb + 2, :],
                                     start=False, stop=True)
                    nc.scalar.activation(hb3[:, eb:eb + 1, :],
                                         psb[:, :], FT.Tanh)

            # --- assemble h_aug.T [KH, R] and its NG zero-padded group copies ---
            # rows 0:H    = hf_used[l,b]  = hT_f block l      -> cols 0:R
            # rows H:2H   = hb_used[l,b]  = hs_b[l+1] block   -> hT_b cols BL:BL+R
            # rows 2H:KH  = onehot(b)
            nc.vector.tensor_copy(out=haug[0:H, :], in_=hT_f[:, 0:R])
            nc.sync.dma_start(out=haug[H:2 * H, :], in_=hT_b[:, BL:BL + R])
            nc.sync.dma_start(out=haug[2 * H:KH, :], in_=hot_d[:, :])
            if cfg.get("mm_f32r"):
                for g in range(NG):
                    nc.vector.memset(lhs_raw[g][:, :], 0.0)
                    nc.sync.dma_start(out=lhs_raw[g][GS * g:GS * g + KH, :],
                                      in_=haug[:, :])
                    # full-tile rounding copy = sole (f32r) producer of lhsg
                    nc.vector.tensor_copy(out=lhsg[g][:, :],
                                          in_=lhs_raw[g][:, :])
            else:
                for g in range(NG):
                    nc.vector.memset(lhsg[g][:, :], 0.0)
                    nc.sync.dma_start(out=lhsg[g][GS * g:GS * g + KH, :],
                                      in_=haug[:, :])
            raw_pool.release()

            # --- projection + log-softmax, two passes, pipelined over row tiles ---
            with tc.tile_pool(name="mpp", bufs=cfg["psum_bufs"], space="PSUM") as mpp, \
                 tc.tile_pool(name="obp", bufs=cfg["out_bufs"]) as obp:

                def mm_pair(ps, t, i, w):
                    g, s = i % NG, i // NG
                    lt = lhsg[g][:, t * 128:(t + 1) * 128]
                    for n0 in range(0, w, MMN):
                        n1 = min(n0 + MMN, w)
                        nc.tensor.matmul(
                            ps[:, n0:n1], lt,
                            resident[:, s * VT + n0:s * VT + n1],
                            start=True, stop=True)

                skip_p1 = cfg.get("skip_pass1")
                skip_p2 = cfg.get("skip_pass2")
                skip_dma = cfg.get("skip_out_dma")
                def emit_p1(ph, i, w):
                    t = ph
                    nm1 = "ps1" if cfg.get("split_psum") else "ps"
                    ps1 = mpp.tile([128, VT], f32, name=nm1,
                                   bufs=2 if cfg.get("split_psum") else None)
                    mm_pair(ps1, t, i, w)
                    nc.scalar.activation(
                        ps1[:, 0:w], ps1[:, 0:w], FT.Exp,
                        accum_out=sums[:, t * NVT + i:t * NVT + i + 1])

                for ph in range((NRT + 1) * cfg.get("repeat", 1)):
                    ph = ph % (NRT + 1)
                    ob = None
                    p2f = cfg.get("p2_first")
                    for i in range(NVT):
                        w = min(VT, VP - i * VT)
                        wo = min(VT, V - i * VT)   # un-padded output width
                        if ph < NRT and not skip_p1 and not p2f:
                            emit_p1(ph, i, w)      # pass 1 for row tile t = ph
                        if ph > 0 and not skip_p2:     # pass 2 for row tile t2 = ph-1
                            t2 = ph - 1
                            nm2 = "ps2" if cfg.get("split_psum") else "ps"
                            ps2 = mpp.tile([128, VT], f32, name=nm2,
                                           bufs=2 if cfg.get("split_psum") else None)
                            mm_pair(ps2, t2, i, w)
                            k = i % OB
                            if k == 0:
                                ob = obp.tile([128, OB * VT], f32, name="ob")
                            nact = cfg.get("fin_act", 0)   # every nact-th on ACT
                            if nact and i % nact == nact - 1:
                                nc.scalar.activation(
                                    ob[:, k * VT:k * VT + w], ps2[:, 0:w],
                                    FT.Identity, bias=Cn_t[:, t2:t2 + 1])
                            else:
                                nc.vector.tensor_scalar_sub(
                                    out=ob[:, k * VT:k * VT + w],
                                    in0=ps2[:, 0:w],
                                    scalar1=C_t[:, t2:t2 + 1])
                            if (k == OB - 1 or i == NVT - 1) and not skip_dma:
                                i0 = i - k
                                bw = k * VT + wo
                                eng = (nc.gpsimd if cfg.get("out_dma_alt")
                                       and (i // OB) % 2 else nc.sync)
                                eng.dma_start(
                                    out=out_d[t2 * 128:(t2 + 1) * 128,
                                              i0 * VT:i0 * VT + bw],
                                    in_=ob[:, 0:bw])
                        if ph < NRT and not skip_p1 and p2f:
                            emit_p1(ph, i, w)
                    if ph < NRT and not skip_p1:  # finish S and log(S) for row tile ph
                        nc.vector.reduce_sum(
                            out=S_t[:, ph:ph + 1],
                            in_=sums[:, ph * NVT:(ph + 1) * NVT], axis=AX.X)
                        nc.scalar.activation(C_t[:, ph:ph + 1],
                                             S_t[:, ph:ph + 1], FT.Ln)
                        if cfg.get("fin_act", 0):
                            nc.vector.tensor_scalar_mul(
                                out=Cn_t[:, ph:ph + 1],
                                in0=C_t[:, ph:ph + 1], scalar1=-1.0)

    nc.compile()
    return nc


def _make_in_maps(cfg, input_ids, we, i2h, h2o, bias, h0f, h0b):
    V = cfg["V"]; EMBD = cfg["EMBD"]; H = cfg["HID"]
    L = cfg["L"]; BL = cfg["BL"]; NC = cfg["ncores"]
    R = L * BL

    ids = np.asarray(input_ids)
    if ids.dtype != np.int32:
        ids = ids.astype(np.int32)
    we = np.ascontiguousarray(np.asarray(we, dtype=np.float32))
    i2h = np.asarray(i2h, dtype=np.float32)
    h2o = np.asarray(h2o, dtype=np.float32)
    bias = np.asarray(bias, dtype=np.float32)
    h0f = np.asarray(h0f, dtype=np.float32)
    h0b = np.asarray(h0b, dtype=np.float32)

    w1 = np.ascontiguousarray(i2h[:EMBD, :])
    w2 = np.ascontiguousarray(i2h[EMBD:, :])
    ident = np.eye(128, dtype=np.float32)
    onehot = np.tile(np.eye(BL, dtype=np.float32), (1, L))  # [BL, R]

    in_maps = []
    for c in range(NC):
        bsl = slice(c * BL, (c + 1) * BL)
        ids_c = np.ascontiguousarray(ids[:, bsl]).reshape(R)       # l-major
        ids_pc = np.ascontiguousarray(ids_c.reshape(R // 128, 128).T)  # [128, NRT]
        projrhs = np.concatenate([h2o, bias[bsl, :]], axis=0)      # [22, V]
        if V % 2:
            # pad vocab to even width (f32r matmul needs even free dims);
            # poison the pad column's bias rows so its logits -> -1e9,
            # exp -> 0, leaving the softmax normalizer unchanged
            pad = np.zeros((projrhs.shape[0], 1), np.float32)
            pad[2 * H:, 0] = -1e9
            projrhs = np.concatenate([projrhs, pad], axis=1)
        projrhs = np.ascontiguousarray(projrhs)
        in_maps.append({
            "ids": ids_pc,
            "we": we,
            "w1": w1,
            "w2": w2,
            "h0ft": np.ascontiguousarray(h0f[bsl, :].T),
            "h0bt": np.ascontiguousarray(h0b[bsl, :].T),
            "projrhs": projrhs,
            "onehot": onehot,
            "ident": ident,
        })
    return in_maps


_CACHE = {}


def _get_nc(cfg_key_and_cfg=None):
    cfg = _default_cfg() if cfg_key_and_cfg is None else cfg_key_and_cfg
    key = tuple(sorted(cfg.items()))
    if key not in _CACHE:
        _CACHE[key] = _build_nc(cfg)
    return _CACHE[key], cfg


def _run(inputs, trace=False, cfg=None):
    from concourse import bass_utils
    nc, cfg = _get_nc(cfg)
    in_maps = _make_in_maps(cfg, **inputs)
    res = bass_utils.run_bass_kernel_spmd(
        nc, in_maps, core_ids=list(range(cfg["ncores"])), trace=trace)
    L, BL, V = cfg["L"], cfg["BL"], cfg["V"]
    out = np.concatenate(
        [r["out"].reshape(L, BL, V) for r in res.results], axis=1)
    return out, res


def kernel(input_ids, we, i2h, h2o, bias, h0f, h0b):
    import os
    trace = bool(os.environ.get("BIRNN_TRACE"))
    out, res = _run(dict(input_ids=input_ids, we=we, i2h=i2h, h2o=h2o,
                         bias=bias, h0f=h0f, h0b=h0b), trace=trace)
    if trace:
        globals()["LAST_RESULTS"] = res
    return out

